# revision 15
# baseline (speedup 1.0000x reference)
"""AdaptiveChebConv (K=3) distributed Bass kernel for 8 TRN2 NeuronCores.

Data-parallel over batch: B=16 -> 2 batches per core. adj/Theta replicated.

Per-core algorithm (per local batch b; N=1024, F=O=64, T=12):
  A  = adj * attn_b                      (DVE elementwise, bf16)
  Z1 = A^T X                             (PE bf16; X natural [n,(f,t)])
  Z2 = A^T Z1                            (PE bf16)
  T[j] = transpose_t(X|Z1|Z2)            (PE transposes -> [f,(t,n)] bf16)
  out[n,o,t] = relu(sum_j Theta_j^T T_j) (PE bf16, K=64 accum x3, ACT relu)
"""
import sys

if "/opt/trn_rl_repo" not in sys.path:
    sys.path.insert(0, "/opt/trn_rl_repo")

import numpy as np
from contextlib import ExitStack

import concourse.bass as bass
import concourse.tile as tile
from concourse import bacc, mybir
from concourse.bass_utils import run_bass_kernel_spmd

N_CORES = 8
B, N, F, T, K, O = 16, 1024, 64, 12, 3, 64
BL = B // N_CORES          # local batches per core = 2
NT = N // 128              # n-tiles = 8
FT = F * T                 # 768
OT = O * T                 # 768
XSPLIT = 4                 # x DMA split (2 n-tiles per chunk)

F32 = mybir.dt.float32
BF16 = mybir.dt.bfloat16

_NC = None


class Ctx:
    pass


def _alloc_xz(cx, name):
    """4 tiles of 2 n-tiles each from the shared 8-slot xz tag."""
    tiles = [
        cx.xz_pool.tile([128, 2 * FT], BF16, tag="xz", bufs=8, name=f"{name}_{i}")
        for i in range(XSPLIT)
    ]

    def sl(mt, lo, size):
        return tiles[mt // 2][:, (mt % 2) * FT + lo: (mt % 2) * FT + lo + size]

    return tiles, sl


def _load_x(cx, b, dep=None):
    tiles, sl = _alloc_xz(cx, f"X{b}")
    last = None
    for xs in range(XSPLIT):
        last = cx.nc.sync.dma_start(
            tiles[xs][:],
            cx.x_ap[b, xs * 256:(xs + 1) * 256].rearrange(
                "(nt p) f t -> p nt (f t)", p=128
            ),
        )
        if dep is not None:
            bass._add_dep_helper(last.ins, dep.ins, True, "dma phase order")
    return sl, last


def _emit_tp_pack(cx, src3, j, nt, g, b, copy_eng=None):
    """One pack: 4 transposes (t = 4g..4g+3) -> psum -> T tile copy."""
    nc = cx.nc
    pt = cx.tp.tile([64, 512], BF16, tag="tp", name="pt")
    for ti in range(4):
        t = 4 * g + ti
        nc.tensor.transpose(
            pt[:, ti * 128:(ti + 1) * 128], src3[:, t, :], cx.ident_t[:]
        )
    if copy_eng is None:
        copy_eng = "v" if (nt + g) % 2 == 0 else "s"
    dst = cx.T_t[(b, j, nt)][:, g * 512:(g + 1) * 512]
    if copy_eng == "v":
        nc.vector.tensor_copy(dst, pt[:])
    else:
        nc.scalar.activation(dst, pt[:], mybir.ActivationFunctionType.Copy)


def _emit_theta_group(cx, b, nt):
    """Theta contraction + relu + out DMA for one n-tile."""
    nc = cx.nc
    o_tile = cx.out_pool.tile([128, OT], F32, tag="out", name="o_tile")
    for (t0, tn) in ((0, 8), (8, 4)):
        pq = cx.qp.tile([128, 512], F32, tag="qp", name="pq")
        for ts in range(tn):
            t = t0 + ts
            for j in range(3):
                nc.tensor.matmul(
                    pq[:, ts * 64:(ts + 1) * 64],
                    cx.T_t[(b, j, nt)][:, t * 128:(t + 1) * 128],
                    cx.theta_t[:, j * 64:(j + 1) * 64],
                    start=(j == 0),
                    stop=(j == 2),
                )
        dst = o_tile[:].rearrange("p (o t) -> p t o", t=T)[:, t0:t0 + tn, :]
        src = pq[:, 0:tn * 64].rearrange("p (t o) -> p t o", o=64)
        nc.scalar.activation(dst, src, mybir.ActivationFunctionType.Relu)
    nc.sync.dma_start(
        cx.out_ap[b, nt * 128:(nt + 1) * 128, :, :].rearrange("p o t -> p (o t)"),
        o_tile[:],
    )


def _emit_A(cx, b, load_adj=False):
    """A(b) = adj * attn[b] as 8 per-mt bf16 tiles (partial-A startup)."""
    nc = cx.nc
    A_t = []
    for mt in range(NT):
        if load_adj:
            adj_t = cx.const_pool.tile(
                [128, 1024], BF16, tag=f"adj{mt}", name=f"adj{mt}"
            )
            nc.sync.dma_start(
                adj_t[:], cx.adj_ap[mt * 128:(mt + 1) * 128, :]
            )
            cx.adj_t.append(adj_t)
        attn_s = cx.scr_pool.tile(
            [128, 1024], BF16, tag="attnscr", bufs=3, name=f"at{b}_{mt}"
        )
        cx.last_attn = nc.sync.dma_start(
            attn_s[:], cx.attn_ap[b, mt * 128:(mt + 1) * 128, :]
        )
        if cx.attn_dep is not None:
            bass._add_dep_helper(
                cx.last_attn.ins, cx.attn_dep.ins, True, "dma phase order"
            )
        a = cx.a_pool.tile([128, 1024], BF16, tag=f"A{mt}", bufs=2, name=f"A{b}_{mt}")
        nc.vector.tensor_mul(a[:], attn_s[:], cx.adj_t[mt][:])
        A_t.append(a)
    return A_t


def _emit_big_matmul(cx, A_t, dst_fn, rhs_fn):
    """dst = A^T rhs (16 psum groups of 8 accumulating MMs)."""
    nc = cx.nc
    for nt in range(NT):
        for ch in range(2):
            pz = cx.zp.tile([128, 384], F32, tag="zp", name="pz")
            for mt in range(NT):
                nc.tensor.matmul(
                    pz[:],
                    A_t[mt][:, nt * 128:(nt + 1) * 128],
                    rhs_fn(mt, ch * 384, 384),
                    start=(mt == 0),
                    stop=(mt == NT - 1),
                )
            nc.vector.tensor_copy(dst_fn(nt, ch * 384, 384), pz[:])


def _emit_batch_main(cx, b, x_sl, A_t):
    Z1 = cx.z_pool.tile([128, NT * FT], BF16, tag="Z1", name=f"Z1_{b}")

    def z1_sl(mt, lo, sz):
        return Z1[:, mt * FT + lo: mt * FT + lo + sz]

    _emit_big_matmul(cx, A_t, z1_sl, x_sl)

    # Z1 transposes (contiguous block)
    for nt in range(NT):
        src3 = Z1[:, nt * FT:(nt + 1) * FT].rearrange("p (f t) -> p t f", t=T)
        for g in range(3):
            _emit_tp_pack(cx, src3, 1, nt, g, b)

    _, z2_sl = _alloc_xz(cx, f"Z2_{b}")
    _emit_big_matmul(cx, A_t, z2_sl, z1_sl)

    for nt in range(NT):
        src3 = z2_sl(nt, 0, FT).rearrange("p (f t) -> p t f", t=T)
        for g in range(3):
            _emit_tp_pack(cx, src3, 2, nt, g, b)
        if nt > 0:
            _emit_theta_group(cx, b, nt - 1)
    _emit_theta_group(cx, b, NT - 1)


def _build():
    nc = bacc.Bacc("TRN2", target_bir_lowering=False, debug=False)
    cx = Ctx()
    cx.nc = nc
    cx.x_ap = nc.dram_tensor("x", [BL, N, F, T], BF16, kind="ExternalInput").ap()
    cx.xt_ap = nc.dram_tensor("xt", [BL, F, T, N], BF16, kind="ExternalInput").ap()
    cx.attn_ap = nc.dram_tensor(
        "spatial_attention", [BL, N, N], BF16, kind="ExternalInput"
    ).ap()
    cx.adj_ap = nc.dram_tensor("adj", [N, N], BF16, kind="ExternalInput").ap()
    cx.theta_ap = nc.dram_tensor("Theta", [K, F, O], BF16, kind="ExternalInput").ap()
    cx.ident_ap = nc.dram_tensor("ident", [128, 128], BF16, kind="ExternalInput").ap()
    cx.out_ap = nc.dram_tensor("out", [BL, N, O, T], F32, kind="ExternalOutput").ap()

    with tile.TileContext(nc) as tc, ExitStack() as ctx:
        cx.a_pool = ctx.enter_context(tc.tile_pool(name="apool", bufs=2))
        cx.xz_pool = ctx.enter_context(tc.tile_pool(name="xz", bufs=1))
        cx.z_pool = ctx.enter_context(tc.tile_pool(name="zpool", bufs=1))
        cx.t_pool = ctx.enter_context(tc.tile_pool(name="tpool", bufs=1))
        cx.scr_pool = ctx.enter_context(tc.tile_pool(name="scr", bufs=3))
        cx.out_pool = ctx.enter_context(tc.tile_pool(name="outp", bufs=3))
        cx.const_pool = ctx.enter_context(tc.tile_pool(name="const", bufs=1))
        cx.zp = ctx.enter_context(tc.tile_pool(name="zp", bufs=3, space="PSUM"))
        cx.tp = ctx.enter_context(tc.tile_pool(name="tp", bufs=3, space="PSUM"))
        cx.qp = ctx.enter_context(tc.tile_pool(name="qp", bufs=2, space="PSUM"))

        cx.ident_t = cx.const_pool.tile([128, 128], BF16, tag="ident")
        nc.sync.dma_start(cx.ident_t[:], cx.ident_ap[:])
        cx.theta_t = cx.const_pool.tile([64, K * O], BF16, tag="theta")
        nc.sync.dma_start(cx.theta_t[:], cx.theta_ap.rearrange("k f o -> f k o"))
        cx.adj_t = []

        # T tiles: (batch, j, nt) -> [64, T*128]; X_T (j=0) double-buffered
        # across batches, j=1,2 single-buffered by shared tag.
        cx.T_t = {}
        for b in range(BL):
            for j in range(3):
                for nt in range(NT):
                    tag = f"T{j}_{nt}" if j > 0 else f"T0_{nt}_{b}"
                    cx.T_t[(b, j, nt)] = cx.t_pool.tile(
                        [64, T * 128], BF16, tag=tag, name=f"T{b}{j}{nt}"
                    )

        # DMA phases: 1) x(b0)+adj+attn(b0) get exclusive early bandwidth;
        # 2) x(b1)+xt(b0) gated on last attn(b0); 3) attn(b1)+xt(b1) gated
        # on phase 2. X_T comes pre-transposed from the host into T0 tiles.
        cx.attn_dep = None
        x_sl0, _ = _load_x(cx, 0)
        A_t0 = _emit_A(cx, 0, load_adj=True)
        ph1 = cx.last_attn
        x_sl1, xlast = _load_x(cx, 1, dep=ph1)
        ph2 = xlast
        for nt in range(NT):
            ph2 = nc.sync.dma_start(
                cx.T_t[(0, 0, nt)][:],
                cx.xt_ap[0, :, :, nt * 128:(nt + 1) * 128],
            )
            bass._add_dep_helper(ph2.ins, ph1.ins, True, "dma phase order")
        for nt in range(NT):
            d = nc.sync.dma_start(
                cx.T_t[(1, 0, nt)][:],
                cx.xt_ap[1, :, :, nt * 128:(nt + 1) * 128],
            )
            bass._add_dep_helper(d.ins, ph2.ins, True, "dma phase order")
        _emit_batch_main(cx, 0, x_sl0, A_t0)
        cx.attn_dep = ph2
        A_t1 = _emit_A(cx, 1)
        _emit_batch_main(cx, 1, x_sl1, A_t1)

    nc.compile()
    return nc


def _make_in_maps(inputs):
    import ml_dtypes

    bf = ml_dtypes.bfloat16
    x = np.ascontiguousarray(np.asarray(inputs["x"], dtype=np.float32).astype(bf))
    xt = np.ascontiguousarray(x.transpose(0, 2, 3, 1))
    attn = np.ascontiguousarray(
        np.asarray(inputs["spatial_attention"], dtype=np.float32).astype(bf)
    )
    adj = np.ascontiguousarray(np.asarray(inputs["adj"], dtype=np.float32).astype(bf))
    theta = np.ascontiguousarray(
        np.asarray(inputs["Theta"], dtype=np.float32).astype(bf)
    )
    ident = np.eye(128, dtype=bf)

    in_maps = []
    for i in range(N_CORES):
        s = slice(i * BL, (i + 1) * BL)
        in_maps.append(
            {
                "x": x[s],
                "xt": xt[s],
                "spatial_attention": attn[s],
                "adj": adj,
                "Theta": theta,
                "ident": ident,
            }
        )
    return in_maps


def kernel(**inputs):
    global _NC
    if _NC is None:
        _NC = _build()
    nc = _NC
    in_maps = _make_in_maps(inputs)
    res = run_bass_kernel_spmd(nc, in_maps, core_ids=list(range(N_CORES)))
    out = np.concatenate([res.results[i]["out"] for i in range(N_CORES)], axis=0)
    return out


# revision 16
# speedup vs baseline: 1.2036x; 1.2036x over previous
"""AdaptiveChebConv (K=3) distributed Bass kernel for 8 TRN2 NeuronCores.

Data-parallel over batch: B=16 -> 2 batches per core. adj/Theta replicated.

Per-core algorithm (per local batch b; N=1024, F=O=64, T=12), using the
commutation of the Theta (feature) contraction with the A (node) hops:

  out = relu(W0 + A^T (W1 + A^T W2)),   W_k[n,o,t] = sum_f X[n,f,t] Theta_k[f,o]

The host supplies X^T (xt = x.transpose(0,2,3,1), bf16) so each W_k is a
K=64 matmul with X^T tiles as the stationary operand -- no on-device
transposes at all. All intermediates in (t,o)-major layout; the final
relu-copy de-swizzles to the natural (o,t) output layout.
"""
import sys

if "/opt/trn_rl_repo" not in sys.path:
    sys.path.insert(0, "/opt/trn_rl_repo")

import numpy as np
from contextlib import ExitStack

import concourse.bass as bass
import concourse.tile as tile
from concourse import bacc, mybir
from concourse.bass_utils import run_bass_kernel_spmd

N_CORES = 8
B, N, F, T, K, O = 16, 1024, 64, 12, 3, 64
BL = B // N_CORES          # local batches per core = 2
NT = N // 128              # n-tiles = 8
FT = F * T                 # 768
OT = O * T                 # 768

F32 = mybir.dt.float32
BF16 = mybir.dt.bfloat16

_NC = None


class Ctx:
    pass


def _emit_A(cx, b, load_adj=False):
    """A(b) = adj * attn[b] as 8 per-mt bf16 tiles."""
    nc = cx.nc
    A_t = []
    for mt in range(NT):
        if load_adj:
            adj_t = cx.const_pool.tile(
                [128, 1024], BF16, tag=f"adj{mt}", name=f"adj{mt}"
            )
            nc.sync.dma_start(adj_t[:], cx.adj_ap[mt * 128:(mt + 1) * 128, :])
            cx.adj_t.append(adj_t)
        attn_s = cx.scr_pool.tile(
            [128, 1024], BF16, tag="attnscr", bufs=3, name=f"at{b}_{mt}"
        )
        d = nc.sync.dma_start(
            attn_s[:], cx.attn_ap[b, mt * 128:(mt + 1) * 128, :]
        )
        if cx.attn_dep is not None:
            bass._add_dep_helper(d.ins, cx.attn_dep.ins, True, "dma phase order")
        cx.last_attn = d
        a = cx.a_pool.tile(
            [128, 1024], BF16, tag=f"A{mt}", bufs=2, name=f"A{b}_{mt}"
        )
        nc.vector.tensor_mul(a[:], attn_s[:], cx.adj_t[mt][:])
        A_t.append(a)
    return A_t


def _emit_theta_mms(cx, pq, b, nt, ch, kk, start, stop):
    """6 accumulating K=64 matmuls: psum[:, ti*64:+64] += T0_slice^T Theta_kk."""
    nc = cx.nc
    for ti in range(6):
        t = ch * 6 + ti
        nc.tensor.matmul(
            pq[:, ti * 64:(ti + 1) * 64],
            cx.T_t[(b, nt)][:, t * 128:(t + 1) * 128],
            cx.theta_t[:, kk * 64:(kk + 1) * 64],
            start=(start and ti == 0),
            stop=(stop and ti == 5),
        )


def _emit_w2(cx, b, W2):
    """W2 = X . Theta_2 in (t,o)-major layout, from host X^T tiles."""
    nc = cx.nc
    for nt in range(NT):
        for ch in range(2):
            pq = cx.zp.tile([128, 384], F32, tag="zp", name="pq")
            _emit_theta_mms(cx, pq, b, nt, ch, 2, True, True)
            nc.vector.tensor_copy(
                W2[:, nt * FT + ch * 384: nt * FT + (ch + 1) * 384], pq[:]
            )


def _emit_hop(cx, b, A_t, rhs_all, kk, dst):
    """dst = A^T rhs_all + X.Theta_kk  (16 psum groups; fused Theta MMs).

    If dst is None this is the output stage: relu-copy to out tiles + DMA.
    """
    nc = cx.nc
    for nt in range(NT):
        o_tile = None
        if dst is None:
            o_tile = cx.out_pool.tile([128, OT], F32, tag="out", name="o_tile")
        for ch in range(2):
            pz = cx.zp.tile([128, 384], F32, tag="zp", name="pz")
            for mt in range(NT):
                nc.tensor.matmul(
                    pz[:],
                    A_t[mt][:, nt * 128:(nt + 1) * 128],
                    rhs_all[:, mt * FT + ch * 384: mt * FT + (ch + 1) * 384],
                    start=(mt == 0),
                    stop=False,
                )
            _emit_theta_mms(cx, pz, b, nt, ch, kk, False, True)
            if dst is not None:
                nc.vector.tensor_copy(
                    dst[:, nt * FT + ch * 384: nt * FT + (ch + 1) * 384], pz[:]
                )
            else:
                d = o_tile[:].rearrange("p (o t) -> p t o", t=T)[
                    :, ch * 6:(ch + 1) * 6, :
                ]
                s = pz[:].rearrange("p (t o) -> p t o", o=64)
                nc.scalar.activation(d, s, mybir.ActivationFunctionType.Relu)
        if dst is None:
            nc.sync.dma_start(
                cx.out_ap[b, nt * 128:(nt + 1) * 128, :, :].rearrange(
                    "p o t -> p (o t)"
                ),
                o_tile[:],
            )


def _build():
    nc = bacc.Bacc("TRN2", target_bir_lowering=False, debug=False)
    cx = Ctx()
    cx.nc = nc
    cx.xt_ap = nc.dram_tensor("xt", [BL, F, T, N], BF16, kind="ExternalInput").ap()
    cx.attn_ap = nc.dram_tensor(
        "spatial_attention", [BL, N, N], BF16, kind="ExternalInput"
    ).ap()
    cx.adj_ap = nc.dram_tensor("adj", [N, N], BF16, kind="ExternalInput").ap()
    cx.theta_ap = nc.dram_tensor("Theta", [K, F, O], BF16, kind="ExternalInput").ap()
    cx.out_ap = nc.dram_tensor("out", [BL, N, O, T], F32, kind="ExternalOutput").ap()

    with tile.TileContext(nc) as tc, ExitStack() as ctx:
        cx.a_pool = ctx.enter_context(tc.tile_pool(name="apool", bufs=2))
        cx.w_pool = ctx.enter_context(tc.tile_pool(name="wpool", bufs=2))
        cx.t_pool = ctx.enter_context(tc.tile_pool(name="tpool", bufs=1))
        cx.scr_pool = ctx.enter_context(tc.tile_pool(name="scr", bufs=3))
        cx.out_pool = ctx.enter_context(tc.tile_pool(name="outp", bufs=3))
        cx.const_pool = ctx.enter_context(tc.tile_pool(name="const", bufs=1))
        cx.zp = ctx.enter_context(tc.tile_pool(name="zp", bufs=7, space="PSUM"))

        cx.theta_t = cx.const_pool.tile([64, K * O], BF16, tag="theta")
        nc.sync.dma_start(cx.theta_t[:], cx.theta_ap.rearrange("k f o -> f k o"))
        cx.adj_t = []
        cx.attn_dep = None

        # T0 tiles: host-transposed X^T per (batch, n-tile): [64, T*128]
        cx.T_t = {}
        for b in range(BL):
            for nt in range(NT):
                cx.T_t[(b, nt)] = cx.t_pool.tile(
                    [64, T * 128], BF16, tag=f"T0_{nt}_{b}", name=f"T{b}_{nt}"
                )
        for b in range(BL):
            for nt in range(NT):
                nc.sync.dma_start(
                    cx.T_t[(b, nt)][:],
                    cx.xt_ap[b, :, :, nt * 128:(nt + 1) * 128],
                )

        # W2 for both batches early (PE filler during attn/adj DMA).
        W2 = [
            cx.w_pool.tile([128, NT * FT], BF16, tag="W2", name=f"W2_{b}")
            for b in range(BL)
        ]
        for b in range(BL):
            _emit_w2(cx, b, W2[b])

        for b in range(BL):
            A_t = _emit_A(cx, b, load_adj=(b == 0))
            if b == 0:
                cx.attn_dep = cx.last_attn  # gate batch-1 attn behind batch-0
            V = cx.w_pool.tile([128, NT * FT], BF16, tag="V", name=f"V_{b}")
            _emit_hop(cx, b, A_t, W2[b], 1, V)
            _emit_hop(cx, b, A_t, V, 0, None)

    nc.compile()
    return nc


def _make_in_maps(inputs):
    import ml_dtypes

    bf = ml_dtypes.bfloat16
    x = np.asarray(inputs["x"], dtype=np.float32).astype(bf)
    xt = np.ascontiguousarray(x.transpose(0, 2, 3, 1))
    attn = np.ascontiguousarray(
        np.asarray(inputs["spatial_attention"], dtype=np.float32).astype(bf)
    )
    adj = np.ascontiguousarray(np.asarray(inputs["adj"], dtype=np.float32).astype(bf))
    theta = np.ascontiguousarray(
        np.asarray(inputs["Theta"], dtype=np.float32).astype(bf)
    )

    in_maps = []
    for i in range(N_CORES):
        s = slice(i * BL, (i + 1) * BL)
        in_maps.append(
            {
                "xt": xt[s],
                "spatial_attention": attn[s],
                "adj": adj,
                "Theta": theta,
            }
        )
    return in_maps


def kernel(**inputs):
    global _NC
    if _NC is None:
        _NC = _build()
    nc = _NC
    in_maps = _make_in_maps(inputs)
    res = run_bass_kernel_spmd(nc, in_maps, core_ids=list(range(N_CORES)))
    out = np.concatenate([res.results[i]["out"] for i in range(N_CORES)], axis=0)
    return out


# revision 17
# speedup vs baseline: 1.2951x; 1.0760x over previous
"""AdaptiveChebConv (K=3) distributed Bass kernel for 8 TRN2 NeuronCores.

Data-parallel over batch: B=16 -> 2 batches per core. adj/Theta replicated.

Per-core algorithm (per local batch b; N=1024, F=O=64, T=12), using the
commutation of the Theta (feature) contraction with the A (node) hops:

  out = relu(W0 + A^T (W1 + A^T W2)),   W_k[n,o,t] = sum_f X[n,f,t] Theta_k[f,o]

The host supplies X^T (xt = x.transpose(0,2,3,1), bf16) so each W_k is a
K=64 matmul with X^T tiles as the stationary operand -- no on-device
transposes at all. All intermediates in (t,o)-major layout; the final
relu-copy de-swizzles to the natural (o,t) output layout.
"""
import sys

if "/opt/trn_rl_repo" not in sys.path:
    sys.path.insert(0, "/opt/trn_rl_repo")

import numpy as np
from contextlib import ExitStack

import concourse.bass as bass
import concourse.tile as tile
from concourse import bacc, mybir
from concourse.bass_utils import run_bass_kernel_spmd

N_CORES = 8
B, N, F, T, K, O = 16, 1024, 64, 12, 3, 64
BL = B // N_CORES          # local batches per core = 2
NT = N // 128              # n-tiles = 8
FT = F * T                 # 768
OT = O * T                 # 768

F32 = mybir.dt.float32
BF16 = mybir.dt.bfloat16

_NC = None


class Ctx:
    pass


def _emit_A(cx, b, load_adj=False):
    """A(b) = adj * attn[b] as 8 per-mt bf16 tiles."""
    nc = cx.nc
    A_t = []
    for mt in range(NT):
        if load_adj:
            adj_t = cx.const_pool.tile(
                [128, 1024], BF16, tag=f"adj{mt}", name=f"adj{mt}"
            )
            nc.sync.dma_start(adj_t[:], cx.adj_ap[mt * 128:(mt + 1) * 128, :])
            cx.adj_t.append(adj_t)
        attn_s = cx.scr_pool.tile(
            [128, 1024], BF16, tag="attnscr", bufs=3, name=f"at{b}_{mt}"
        )
        d = nc.sync.dma_start(
            attn_s[:], cx.attn_ap[b, mt * 128:(mt + 1) * 128, :]
        )
        if cx.attn_dep is not None:
            bass._add_dep_helper(d.ins, cx.attn_dep.ins, True, "dma phase order")
        cx.last_attn = d
        a = cx.a_pool.tile(
            [128, 1024], BF16, tag=f"A{mt}", bufs=2, name=f"A{b}_{mt}"
        )
        nc.vector.tensor_mul(a[:], attn_s[:], cx.adj_t[mt][:])
        A_t.append(a)
    return A_t


def _emit_theta_mms(cx, pq, b, nt, ch, kk, start, stop):
    """6 accumulating K=64 matmuls: psum[:, ti*64:+64] += T0_slice^T Theta_kk."""
    nc = cx.nc
    for ti in range(6):
        t = ch * 6 + ti
        nc.tensor.matmul(
            pq[:, ti * 64:(ti + 1) * 64],
            cx.T_t[(b, nt)][:, t * 128:(t + 1) * 128],
            cx.theta_t[:, kk * 64:(kk + 1) * 64],
            start=(start and ti == 0),
            stop=(stop and ti == 5),
        )


def _emit_w2(cx, b, W2):
    """W2 = X . Theta_2 in (t,o)-major layout, from host X^T tiles."""
    nc = cx.nc
    for nt in range(NT):
        for ch in range(2):
            pq = cx.zp.tile([128, 384], F32, tag="zp", name="pq")
            _emit_theta_mms(cx, pq, b, nt, ch, 2, True, True)
            nc.vector.tensor_copy(
                W2[:, nt * FT + ch * 384: nt * FT + (ch + 1) * 384], pq[:]
            )


def _emit_hop(cx, b, A_t, rhs_all, kk, dst):
    """dst = A^T rhs_all + X.Theta_kk  (16 psum groups; fused Theta MMs).

    If dst is None this is the output stage: relu-copy to out tiles + DMA.
    """
    nc = cx.nc
    for nt in range(NT):
        o_tile = None
        if dst is None:
            o_tile = cx.out_pool.tile([128, OT], F32, tag="out", name="o_tile")
        for ch in range(2):
            pz = cx.zp.tile([128, 384], F32, tag="zp", name="pz")
            _emit_theta_mms(cx, pz, b, nt, ch, kk, True, False)
            for mt in range(NT):
                nc.tensor.matmul(
                    pz[:],
                    A_t[mt][:, nt * 128:(nt + 1) * 128],
                    rhs_all[:, mt * FT + ch * 384: mt * FT + (ch + 1) * 384],
                    start=False,
                    stop=(mt == NT - 1),
                )
            if dst is not None:
                nc.vector.tensor_copy(
                    dst[:, nt * FT + ch * 384: nt * FT + (ch + 1) * 384], pz[:]
                )
            else:
                d = o_tile[:].rearrange("p (o t) -> p t o", t=T)[
                    :, ch * 6:(ch + 1) * 6, :
                ]
                s = pz[:].rearrange("p (t o) -> p t o", o=64)
                nc.scalar.activation(d, s, mybir.ActivationFunctionType.Relu)
        if dst is None:
            nc.sync.dma_start(
                cx.out_ap[b, nt * 128:(nt + 1) * 128, :, :].rearrange(
                    "p o t -> p (o t)"
                ),
                o_tile[:],
            )


def _build():
    nc = bacc.Bacc("TRN2", target_bir_lowering=False, debug=False)
    cx = Ctx()
    cx.nc = nc
    cx.xt_ap = nc.dram_tensor("xt", [BL, NT, F, T, 128], BF16, kind="ExternalInput").ap()
    cx.attn_ap = nc.dram_tensor(
        "spatial_attention", [BL, N, N], BF16, kind="ExternalInput"
    ).ap()
    cx.adj_ap = nc.dram_tensor("adj", [N, N], BF16, kind="ExternalInput").ap()
    cx.theta_ap = nc.dram_tensor("Theta", [K, F, O], BF16, kind="ExternalInput").ap()
    cx.out_ap = nc.dram_tensor("out", [BL, N, O, T], F32, kind="ExternalOutput").ap()

    with tile.TileContext(nc) as tc, ExitStack() as ctx:
        cx.a_pool = ctx.enter_context(tc.tile_pool(name="apool", bufs=2))
        cx.w_pool = ctx.enter_context(tc.tile_pool(name="wpool", bufs=2))
        cx.t_pool = ctx.enter_context(tc.tile_pool(name="tpool", bufs=1))
        cx.scr_pool = ctx.enter_context(tc.tile_pool(name="scr", bufs=3))
        cx.out_pool = ctx.enter_context(tc.tile_pool(name="outp", bufs=3))
        cx.const_pool = ctx.enter_context(tc.tile_pool(name="const", bufs=1))
        cx.zp = ctx.enter_context(tc.tile_pool(name="zp", bufs=7, space="PSUM"))

        cx.theta_t = cx.const_pool.tile([64, K * O], BF16, tag="theta")
        nc.sync.dma_start(cx.theta_t[:], cx.theta_ap.rearrange("k f o -> f k o"))
        cx.adj_t = []
        cx.attn_dep = None

        # T0 tiles: host-transposed X^T per (batch, n-tile): [64, T*128]
        cx.T_t = {}
        for b in range(BL):
            for nt in range(NT):
                cx.T_t[(b, nt)] = cx.t_pool.tile(
                    [64, T * 128], BF16, tag=f"T0_{nt}_{b}", name=f"T{b}_{nt}"
                )
        for b in range(BL):
            for nt in range(NT):
                nc.sync.dma_start(
                    cx.T_t[(b, nt)][:],
                    cx.xt_ap[b, nt].rearrange("f t n -> f (t n)"),
                )

        # W2 for both batches early (PE filler during attn/adj DMA).
        W2 = [
            cx.w_pool.tile([128, NT * FT], BF16, tag="W2", name=f"W2_{b}")
            for b in range(BL)
        ]
        for b in range(BL):
            _emit_w2(cx, b, W2[b])

        for b in range(BL):
            A_t = _emit_A(cx, b, load_adj=(b == 0))
            if b == 0:
                cx.attn_dep = cx.last_attn  # gate batch-1 attn behind batch-0
            V = cx.w_pool.tile([128, NT * FT], BF16, tag="V", name=f"V_{b}")
            _emit_hop(cx, b, A_t, W2[b], 1, V)
            _emit_hop(cx, b, A_t, V, 0, None)

    nc.compile()
    return nc


def _make_in_maps(inputs):
    import ml_dtypes

    bf = ml_dtypes.bfloat16
    x = np.asarray(inputs["x"], dtype=np.float32).astype(bf)
    # [B, N, F, T] -> [B, NT, F, T, 128] so each (batch, n-tile) block is
    # one contiguous 192KB DMA
    xt = np.ascontiguousarray(
        x.reshape(B, NT, 128, F, T).transpose(0, 1, 3, 4, 2)
    )
    attn = np.ascontiguousarray(
        np.asarray(inputs["spatial_attention"], dtype=np.float32).astype(bf)
    )
    adj = np.ascontiguousarray(np.asarray(inputs["adj"], dtype=np.float32).astype(bf))
    theta = np.ascontiguousarray(
        np.asarray(inputs["Theta"], dtype=np.float32).astype(bf)
    )

    in_maps = []
    for i in range(N_CORES):
        s = slice(i * BL, (i + 1) * BL)
        in_maps.append(
            {
                "xt": xt[s],
                "spatial_attention": attn[s],
                "adj": adj,
                "Theta": theta,
            }
        )
    return in_maps


def kernel(**inputs):
    global _NC
    if _NC is None:
        _NC = _build()
    nc = _NC
    in_maps = _make_in_maps(inputs)
    res = run_bass_kernel_spmd(nc, in_maps, core_ids=list(range(N_CORES)))
    out = np.concatenate([res.results[i]["out"] for i in range(N_CORES)], axis=0)
    return out


# revision 19
# speedup vs baseline: 1.3236x; 1.0221x over previous
"""AdaptiveChebConv (K=3) distributed Bass kernel for 8 TRN2 NeuronCores.

Data-parallel over batch: B=16 -> 2 batches per core. adj/Theta replicated.

Per-core algorithm (per local batch b; N=1024, F=O=64, T=12), using the
commutation of the Theta (feature) contraction with the A (node) hops:

  out = relu(W0 + A^T (W1 + A^T W2)),   W_k[n,o,t] = sum_f X[n,f,t] Theta_k[f,o]

The host supplies X^T (xt = x.transpose(0,2,3,1), bf16) so each W_k is a
K=64 matmul with X^T tiles as the stationary operand -- no on-device
transposes at all. All intermediates in (t,o)-major layout; the final
relu-copy de-swizzles to the natural (o,t) output layout.
"""
import sys

if "/opt/trn_rl_repo" not in sys.path:
    sys.path.insert(0, "/opt/trn_rl_repo")

import numpy as np
from contextlib import ExitStack

import concourse.bass as bass
import concourse.tile as tile
from concourse import bacc, mybir
from concourse.bass_utils import run_bass_kernel_spmd

N_CORES = 8
B, N, F, T, K, O = 16, 1024, 64, 12, 3, 64
BL = B // N_CORES          # local batches per core = 2
NT = N // 128              # n-tiles = 8
FT = F * T                 # 768
OT = O * T                 # 768

F32 = mybir.dt.float32
BF16 = mybir.dt.bfloat16

_NC = None


class Ctx:
    pass


def _emit_A(cx, b, load_adj=False):
    """A(b) = adj * attn[b] as 8 per-mt bf16 tiles."""
    nc = cx.nc
    A_t = []
    for mt in range(NT):
        if load_adj:
            adj_t = cx.const_pool.tile(
                [128, 1024], BF16, tag=f"adj{mt}", name=f"adj{mt}"
            )
            da = nc.sync.dma_start(
                adj_t[:], cx.adj_ap[mt * 128:(mt + 1) * 128, :]
            )
            if cx.attn_dep is not None:
                bass._add_dep_helper(
                    da.ins, cx.attn_dep.ins, True, "dma phase order"
                )
            cx.adj_t.append(adj_t)
        attn_s = cx.scr_pool.tile(
            [128, 1024], BF16, tag="attnscr", bufs=3, name=f"at{b}_{mt}"
        )
        d = nc.sync.dma_start(
            attn_s[:], cx.attn_ap[b, mt * 128:(mt + 1) * 128, :]
        )
        if cx.attn_dep is not None:
            bass._add_dep_helper(d.ins, cx.attn_dep.ins, True, "dma phase order")
        cx.last_attn = d
        a = cx.a_pool.tile(
            [128, 1024], BF16, tag=f"A{mt}", bufs=2, name=f"A{b}_{mt}"
        )
        nc.vector.tensor_mul(a[:], attn_s[:], cx.adj_t[mt][:])
        A_t.append(a)
    return A_t


def _emit_theta_mms(cx, pq, b, nt, ch, kk, start, stop):
    """6 accumulating K=64 matmuls: psum[:, ti*64:+64] += T0_slice^T Theta_kk."""
    nc = cx.nc
    for ti in range(6):
        t = ch * 6 + ti
        nc.tensor.matmul(
            pq[:, ti * 64:(ti + 1) * 64],
            cx.T_t[(b, nt)][:, t * 128:(t + 1) * 128],
            cx.theta_t[:, kk * 64:(kk + 1) * 64],
            start=(start and ti == 0),
            stop=(stop and ti == 5),
        )


def _emit_w2(cx, b, W2):
    """W2 = X . Theta_2 in (t,o)-major layout, from host X^T tiles."""
    nc = cx.nc
    for nt in range(NT):
        pqs = []
        for ch in range(2):
            pq = cx.zp.tile([128, 384], F32, tag="zp", name="pq")
            _emit_theta_mms(cx, pq, b, nt, ch, 2, True, True)
            pqs.append(pq)
        for ch in range(2):
            nc.vector.tensor_copy(
                W2[:, nt * FT + ch * 384: nt * FT + (ch + 1) * 384], pqs[ch]
            )


def _emit_hop(cx, b, A_t, rhs_all, kk, dst):
    """dst = A^T rhs_all + X.Theta_kk  (16 psum groups; fused Theta MMs).

    If dst is None this is the output stage: relu-copy to out tiles + DMA.
    """
    nc = cx.nc
    for nt in range(NT):
        o_tile = None
        if dst is None:
            o_tile = cx.out_pool.tile([128, OT], F32, tag="out", name="o_tile")
        pzs = []
        for ch in range(2):
            pz = cx.zp.tile([128, 384], F32, tag="zp", name="pz")
            _emit_theta_mms(cx, pz, b, nt, ch, kk, True, False)
            pzs.append(pz)
        for ch in range(2):
            pz = pzs[ch]
            for mt in range(NT):
                nc.tensor.matmul(
                    pz[:],
                    A_t[mt][:, nt * 128:(nt + 1) * 128],
                    rhs_all[:, mt * FT + ch * 384: mt * FT + (ch + 1) * 384],
                    start=False,
                    stop=(mt == NT - 1),
                )
            if dst is not None:
                nc.vector.tensor_copy(
                    dst[:, nt * FT + ch * 384: nt * FT + (ch + 1) * 384], pz[:]
                )
            else:
                d = o_tile[:].rearrange("p (o t) -> p t o", t=T)[
                    :, ch * 6:(ch + 1) * 6, :
                ]
                s = pz[:].rearrange("p (t o) -> p t o", o=64)
                nc.scalar.activation(d, s, mybir.ActivationFunctionType.Relu)
        if dst is None:
            nc.sync.dma_start(
                cx.out_ap[b, nt * 128:(nt + 1) * 128, :, :].rearrange(
                    "p o t -> p (o t)"
                ),
                o_tile[:],
            )


def _build():
    nc = bacc.Bacc("TRN2", target_bir_lowering=False, debug=False)
    cx = Ctx()
    cx.nc = nc
    cx.xt_ap = nc.dram_tensor("xt", [BL, NT, F, T, 128], BF16, kind="ExternalInput").ap()
    cx.attn_ap = nc.dram_tensor(
        "spatial_attention", [BL, N, N], BF16, kind="ExternalInput"
    ).ap()
    cx.adj_ap = nc.dram_tensor("adj", [N, N], BF16, kind="ExternalInput").ap()
    cx.theta_ap = nc.dram_tensor("Theta", [K, F, O], BF16, kind="ExternalInput").ap()
    cx.out_ap = nc.dram_tensor("out", [BL, N, O, T], F32, kind="ExternalOutput").ap()

    with tile.TileContext(nc) as tc, ExitStack() as ctx:
        cx.a_pool = ctx.enter_context(tc.tile_pool(name="apool", bufs=2))
        cx.w_pool = ctx.enter_context(tc.tile_pool(name="wpool", bufs=2))
        cx.t_pool = ctx.enter_context(tc.tile_pool(name="tpool", bufs=1))
        cx.scr_pool = ctx.enter_context(tc.tile_pool(name="scr", bufs=3))
        cx.out_pool = ctx.enter_context(tc.tile_pool(name="outp", bufs=3))
        cx.const_pool = ctx.enter_context(tc.tile_pool(name="const", bufs=1))
        cx.zp = ctx.enter_context(tc.tile_pool(name="zp", bufs=7, space="PSUM"))

        cx.theta_t = cx.const_pool.tile([64, K * O], BF16, tag="theta")
        nc.sync.dma_start(cx.theta_t[:], cx.theta_ap.rearrange("k f o -> f k o"))
        cx.adj_t = []
        cx.attn_dep = None

        # T0 tiles: host-transposed X^T per (batch, n-tile): [64, T*128]
        cx.T_t = {}
        for b in range(BL):
            for nt in range(NT):
                cx.T_t[(b, nt)] = cx.t_pool.tile(
                    [64, T * 128], BF16, tag=f"T0_{nt}_{b}", name=f"T{b}_{nt}"
                )
        for b in range(BL):
            for nt in range(NT):
                cx.attn_dep = nc.sync.dma_start(
                    cx.T_t[(b, nt)][:],
                    cx.xt_ap[b, nt].rearrange("f t n -> f (t n)"),
                )

        # W2 for both batches early (PE filler during attn/adj DMA).
        W2 = [
            cx.w_pool.tile([128, NT * FT], BF16, tag="W2", name=f"W2_{b}")
            for b in range(BL)
        ]
        for b in range(BL):
            _emit_w2(cx, b, W2[b])

        for b in range(BL):
            A_t = _emit_A(cx, b, load_adj=(b == 0))
            if b == 0:
                cx.attn_dep = cx.last_attn  # gate batch-1 attn behind batch-0
            V = cx.w_pool.tile([128, NT * FT], BF16, tag="V", name=f"V_{b}")
            _emit_hop(cx, b, A_t, W2[b], 1, V)
            _emit_hop(cx, b, A_t, V, 0, None)

    nc.compile()
    return nc


def _make_in_maps(inputs):
    import ml_dtypes

    bf = ml_dtypes.bfloat16
    x = np.asarray(inputs["x"], dtype=np.float32).astype(bf)
    # [B, N, F, T] -> [B, NT, F, T, 128] so each (batch, n-tile) block is
    # one contiguous 192KB DMA
    xt = np.ascontiguousarray(
        x.reshape(B, NT, 128, F, T).transpose(0, 1, 3, 4, 2)
    )
    attn = np.ascontiguousarray(
        np.asarray(inputs["spatial_attention"], dtype=np.float32).astype(bf)
    )
    adj = np.ascontiguousarray(np.asarray(inputs["adj"], dtype=np.float32).astype(bf))
    theta = np.ascontiguousarray(
        np.asarray(inputs["Theta"], dtype=np.float32).astype(bf)
    )

    in_maps = []
    for i in range(N_CORES):
        s = slice(i * BL, (i + 1) * BL)
        in_maps.append(
            {
                "xt": xt[s],
                "spatial_attention": attn[s],
                "adj": adj,
                "Theta": theta,
            }
        )
    return in_maps


def kernel(**inputs):
    global _NC
    if _NC is None:
        _NC = _build()
    nc = _NC
    in_maps = _make_in_maps(inputs)
    res = run_bass_kernel_spmd(nc, in_maps, core_ids=list(range(N_CORES)))
    out = np.concatenate([res.results[i]["out"] for i in range(N_CORES)], axis=0)
    return out
